# revision 18
# baseline (speedup 1.0000x reference)
"""Birman-Schwinger core: K[b] = diag(sqrt|V_b|) @ R_0 @ diag(sqrt|V_b|).

Rank-2 factorization: with g[b,u] = sqrt(|V[b,u]| + eps) / (1 + u) and
d = u - v,

    K[b,u,v] = sign(d) * g_u g_v * (-0.5 sin 2d + 0.5j cos 2d)
             = sign(d) * (L0(u) R0(v) + L1(u) R1(v))        [re/im interleaved]

with L0 = 0.5 g_u sin 2u, L1 = 0.5 g_u cos 2u and R0/R1 the matching
cos/sin basis with g_v folded in.  So every (128, 256)-complex output
tile is a K=2 matmul; inputs are split hi+lo in bf16 (K=6 after the
split, dropping the lo*lo term) to recover ~fp32 accuracy in the fp32
PSUM accumulator.  sign(d) is +1/-1 per tile except the diagonal
tiles, which get an elementwise +-1/0 mask multiply on the Vector
engine while draining PSUM.

No big table is ever read from HBM: per-core input bytes are ~0.7 MB
vs a 64 MB output, so the kernel runs at the DMA-fabric store roofline
(~420 GB/s/core: all 16 SDMA engines pinned at their ~26.5 GB/s cap).
Tensor engine generates tiles into PSUM (256 small matmuls at ~430 ns
each, LDWEIGHTS hidden by the PE reorder window), Vector + Scalar
drain PSUM into ~1 MB SBUF tiles (greedy load-balanced), and stores
alternate between the Sync and Scalar HWDGE rings.  Input loads are
staged so the first matmul starts as soon as the NEFF preamble ends.

Sharding: 8 cores; core c handles batch b = c // 2 and the INTERLEAVED
row blocks 2t + h (h = c % 2, t = 0..15) of the 32 128-row blocks.
The interleaving makes the diagonal land in local chunk m == t on
every core, so one SPMD program serves both halves; the per-core
diagonal mask (band in first vs second half of the chunk) is input
data.  Output is written as interleaved re/im f32 pairs so the
per-core (2048, 8192) f32 result is the complex64 memory layout.
"""

import numpy as np

B = 4
N = 4096
NCORES = 8
HALF = N // 2            # rows per core
P = 128                  # SBUF partitions
NBLK = HALF // P         # 16 row blocks per core
EPS = 1e-10
CF = 512                 # f32 cols per matmul chunk (one PSUM bank)
NCHUNK = (2 * N) // CF   # 16 chunks per row block
TPB = 4                  # chunks per store tile (1 MB)
K6 = 6                   # contraction rows after bf16 hi/lo split

_PROGRAM_CACHE = {}


def _build_program():
    import concourse.bacc as bacc
    import concourse.mybir as mybir
    from concourse.tile import TileContext

    nc = bacc.Bacc("TRN2", target_bir_lowering=False, debug=False)
    lp = nc.dram_tensor("t_lp", [K6, HALF], mybir.dt.bfloat16, kind="ExternalInput").ap()
    ln = nc.dram_tensor("t_ln", [K6, HALF], mybir.dt.bfloat16, kind="ExternalInput").ap()
    rr = nc.dram_tensor("t_r", [K6, 2 * N], mybir.dt.bfloat16, kind="ExternalInput").ap()
    mk = nc.dram_tensor("t_mk", [P, CF], mybir.dt.float32, kind="ExternalInput").ap()
    out = nc.dram_tensor(
        "t_out", [HALF, 2 * N], mybir.dt.float32, kind="ExternalOutput"
    ).ap()
    mult = mybir.AluOpType.mult

    with TileContext(nc) as tc:
        with tc.tile_pool(name="const", bufs=1) as cpool:
            lp_sb = cpool.tile([K6, HALF], mybir.dt.bfloat16)
            ln_sb = cpool.tile([K6, HALF], mybir.dt.bfloat16)
            r_sb = cpool.tile([K6, 2 * N], mybir.dt.bfloat16)
            mk_sb = cpool.tile([P, CF], mybir.dt.float32)
            # The first matmuls (block t=15, chunks 0..3) need only small
            # slices of lp and r: land those first as tiny single-packet
            # DMAs on the otherwise-idle Sync ring so the pipeline starts
            # ~3us earlier.  Everything else follows on the Scalar ring in
            # consumption order (Sync stays clear for the first stores).
            nc.sync.dma_start(
                out=lp_sb[:, 14 * P :], in_=lp[:, 14 * P :], single_packet=True
            )
            nc.scalar.dma_start(
                out=r_sb[:, : TPB * CF], in_=rr[:, : TPB * CF], single_packet=True
            )
            nc.sync.dma_start(
                out=r_sb[:, TPB * CF : 2 * TPB * CF],
                in_=rr[:, TPB * CF : 2 * TPB * CF],
            )
            nc.scalar.dma_start(
                out=r_sb[:, 2 * TPB * CF :], in_=rr[:, 2 * TPB * CF :]
            )
            nc.scalar.dma_start(out=lp_sb[:, : 14 * P], in_=lp[:, : 14 * P])
            nc.scalar.dma_start(out=ln_sb[:, :], in_=ln[:, :])
            nc.scalar.dma_start(out=mk_sb[:, :], in_=mk[:, :])

            with (
                tc.tile_pool(name="psum", bufs=8, space="PSUM") as ppool,
                tc.tile_pool(name="out", bufs=8) as opool,
            ):
                # Greedy busy-ns balancing between the two drain engines
                # (Scalar also pays for the store dispatches on its ring).
                busy = {"v": 0.0, "s": 0.0}
                DRAIN_NS, DISPATCH_NS = 686.0, 610.0
                busy["s"] += 5 * DISPATCH_NS  # const loads above
                ti = 0   # store-ring toggle
                for t in reversed(range(NBLK)):
                    kc = t * P          # local row offset of this block
                    # Narrow tiles at the very start (stores begin sooner,
                    # rings ramp faster) and at the very end (shorter tail).
                    if t == NBLK - 1:
                        widths = (1, 1, 2, 4, 4, 4)
                    elif t == 0:
                        widths = (8, 4, 2, 1, 1)
                    else:
                        widths = (8, 8)
                    cc0 = 0
                    for w in widths:
                        ot = opool.tile([P, w * CF], mybir.dt.float32)
                        for j in range(w):
                            cc = cc0 + j
                            pt = ppool.tile([P, CF], mybir.dt.float32)
                            lhs_sb = lp_sb if cc <= t else ln_sb
                            nc.tensor.matmul(
                                out=pt[:, :],
                                lhsT=lhs_sb[:, kc : kc + P],
                                rhs=r_sb[:, CF * cc : CF * (cc + 1)],
                                start=True,
                                stop=True,
                            )
                            dst = ot[:, CF * j : CF * (j + 1)]
                            if cc == t:
                                nc.vector.tensor_tensor(
                                    out=dst, in0=pt[:, :], in1=mk_sb[:, :], op=mult
                                )
                                busy["v"] += DRAIN_NS
                            elif busy["s"] <= busy["v"]:
                                nc.scalar.copy(out=dst, in_=pt[:, :])
                                busy["s"] += DRAIN_NS
                            else:
                                nc.vector.tensor_copy(out=dst, in_=pt[:, :])
                                busy["v"] += DRAIN_NS
                        if ti % 2 == 0:
                            ring = nc.sync
                        else:
                            ring = nc.scalar
                            busy["s"] += DISPATCH_NS
                        ring.dma_start(
                            out=out[kc : kc + P, CF * cc0 : CF * (cc0 + w)],
                            in_=ot[:, :],
                        )
                        ti += 1
                        cc0 += w
    nc.compile()
    return nc


def _get_program():
    if "nc" not in _PROGRAM_CACHE:
        _PROGRAM_CACHE["nc"] = _build_program()
    return _PROGRAM_CACHE["nc"]


def _host_tables(V):
    """Per-core input arrays."""
    import ml_dtypes

    bf16 = ml_dtypes.bfloat16

    pos = np.arange(N, dtype=np.float64)
    g = np.sqrt(np.abs(V).astype(np.float64) + EPS) / (1.0 + pos)  # (B, N)
    s2, c2 = np.sin(2.0 * pos), np.cos(2.0 * pos)

    def split(x):
        h = x.astype(bf16)
        l = (x - h.astype(np.float64)).astype(bf16)
        return h, l

    # Diagonal-band masks.  h=0 cores (even global blocks) have the band in
    # the first 128 complex cols of their diagonal chunk, h=1 in the second.
    p = np.arange(P)[:, None]
    jj = np.arange(P)[None, :]
    band = np.sign(p - jj).astype(np.float32)        # (128, 128)
    mask_e = np.empty((P, CF), dtype=np.float32)
    mask_o = np.empty((P, CF), dtype=np.float32)
    mask_e[:, 0:256:2] = band
    mask_e[:, 1:256:2] = band
    mask_e[:, 256:] = -1.0
    mask_o[:, 0:256] = 1.0
    mask_o[:, 256:512:2] = band
    mask_o[:, 257:512:2] = band

    in_maps = []
    for c in range(NCORES):
        b, h = divmod(c, 2)
        gb = g[b]
        # v-side basis, interleaved re/im, g_v folded in
        R0 = np.empty(2 * N); R1 = np.empty(2 * N)
        R0[0::2] = -gb * c2
        R0[1::2] = gb * s2
        R1[0::2] = gb * s2
        R1[1::2] = gb * c2
        R0h, R0l = split(R0)
        R1h, R1l = split(R1)
        rhs = np.ascontiguousarray(np.stack([R0h, R0l, R0h, R1h, R1l, R1h]))
        # u-side: this core's interleaved row blocks (global blocks 2t + h)
        ui = np.arange(N).reshape(2 * NBLK, P)[h::2].ravel()
        L0 = 0.5 * gb[ui] * s2[ui]
        L1 = 0.5 * gb[ui] * c2[ui]
        L0h, L0l = split(L0)
        L1h, L1l = split(L1)
        lpos = np.ascontiguousarray(np.stack([L0h, L0h, L0l, L1h, L1h, L1l]))
        lneg = np.ascontiguousarray(-lpos.astype(np.float32)).astype(bf16)
        in_maps.append(
            {
                "t_lp": lpos,
                "t_ln": lneg,
                "t_r": rhs,
                "t_mk": mask_e if h == 0 else mask_o,
            }
        )
    return in_maps


def _run(in_maps, trace=False, **kwargs):
    from concourse import bass_utils

    nc = _get_program()
    return bass_utils.run_bass_kernel_spmd(
        nc, in_maps, core_ids=list(range(NCORES)), trace=trace, **kwargs
    )


def kernel(V):
    V = np.asarray(V, dtype=np.float32)
    assert V.shape == (B, N), V.shape
    in_maps = _host_tables(V)
    res = _run(in_maps, trace=False)
    out = np.empty((B, N, N), dtype=np.complex64)
    for c in range(NCORES):
        b, h = divmod(c, 2)
        plane = np.ascontiguousarray(res.results[c]["t_out"])
        out[b].reshape(2 * NBLK, P, N)[h::2] = plane.view(np.complex64).reshape(
            NBLK, P, N
        )
    return out


# revision 19
# speedup vs baseline: 1.0982x; 1.0982x over previous
"""Birman-Schwinger core: K[b] = diag(sqrt|V_b|) @ R_0 @ diag(sqrt|V_b|).

Rank-2 factorization: with g[b,u] = sqrt(|V[b,u]| + eps) / (1 + u) and
d = u - v,

    K[b,u,v] = sign(d) * g_u g_v * (-0.5 sin 2d + 0.5j cos 2d)
             = sign(d) * (L0(u) R0(v) + L1(u) R1(v))        [re/im interleaved]

with L0 = 0.5 g_u sin 2u, L1 = 0.5 g_u cos 2u and R0/R1 the matching
cos/sin basis with g_v folded in.  So every (128, 256)-complex output
tile is a K=2 matmul; inputs are split hi+lo in bf16 (K=6 after the
split, dropping the lo*lo term) to recover ~fp32 accuracy in the fp32
PSUM accumulator.  sign(d) is +1/-1 per tile except the diagonal
tiles, which get an elementwise +-1/0 mask multiply on the Vector
engine while draining PSUM.

No big table is ever read from HBM: per-core input bytes are ~0.7 MB
vs a 64 MB output, so the kernel runs at the DMA-fabric store roofline
(~420 GB/s/core: all 16 SDMA engines pinned at their ~26.5 GB/s cap).
Tensor engine generates tiles into PSUM (256 small matmuls at ~430 ns
each, LDWEIGHTS hidden by the PE reorder window), Vector + Scalar
drain PSUM into ~1 MB SBUF tiles (greedy load-balanced), and stores
alternate between the Sync and Scalar HWDGE rings.  Input loads are
staged so the first matmul starts as soon as the NEFF preamble ends.

Sharding: 8 cores; core c handles batch b = c // 2 and the INTERLEAVED
row blocks 2t + h (h = c % 2, t = 0..15) of the 32 128-row blocks.
The interleaving makes the diagonal land in local chunk m == t on
every core, so one SPMD program serves both halves; the per-core
diagonal mask (band in first vs second half of the chunk) is input
data.  Output is written as interleaved re/im f32 pairs so the
per-core (2048, 8192) f32 result is the complex64 memory layout.
"""

import numpy as np

B = 4
N = 4096
NCORES = 8
HALF = N // 2            # rows per core
P = 128                  # SBUF partitions
NBLK = HALF // P         # 16 row blocks per core
EPS = 1e-10
CF = 512                 # f32 cols per matmul chunk (one PSUM bank)
NCHUNK = (2 * N) // CF   # 16 chunks per row block
TPB = 4                  # chunks per store tile (1 MB)
K6 = 6                   # contraction rows after bf16 hi/lo split

_PROGRAM_CACHE = {}


def _build_program():
    import concourse.bacc as bacc
    import concourse.mybir as mybir
    from concourse.tile import TileContext

    nc = bacc.Bacc("TRN2", target_bir_lowering=False, debug=False)
    lp = nc.dram_tensor("t_lp", [K6, HALF], mybir.dt.bfloat16, kind="ExternalInput").ap()
    ln = nc.dram_tensor("t_ln", [K6, HALF], mybir.dt.bfloat16, kind="ExternalInput").ap()
    rr = nc.dram_tensor("t_r", [K6, 2 * N], mybir.dt.bfloat16, kind="ExternalInput").ap()
    mk = nc.dram_tensor("t_mk", [P, CF], mybir.dt.float32, kind="ExternalInput").ap()
    out = nc.dram_tensor(
        "t_out", [HALF, 2 * N], mybir.dt.float32, kind="ExternalOutput"
    ).ap()
    mult = mybir.AluOpType.mult

    with TileContext(nc) as tc:
        with tc.tile_pool(name="const", bufs=1) as cpool:
            lp_sb = cpool.tile([K6, HALF], mybir.dt.bfloat16)
            ln_sb = cpool.tile([K6, HALF], mybir.dt.bfloat16)
            r_sb = cpool.tile([K6, 2 * N], mybir.dt.bfloat16)
            mk_sb = cpool.tile([P, CF], mybir.dt.float32)
            # The first matmuls (block t=15, chunks 0..3) need only small
            # slices of lp and r: land those first as tiny single-packet
            # DMAs on the otherwise-idle Sync ring so the pipeline starts
            # ~3us earlier.  Everything else follows on the Scalar ring in
            # consumption order (Sync stays clear for the first stores).
            nc.sync.dma_start(
                out=lp_sb[:, 14 * P :], in_=lp[:, 14 * P :], single_packet=True
            )
            nc.scalar.dma_start(
                out=r_sb[:, : TPB * CF], in_=rr[:, : TPB * CF], single_packet=True
            )
            nc.sync.dma_start(
                out=r_sb[:, TPB * CF : 2 * TPB * CF],
                in_=rr[:, TPB * CF : 2 * TPB * CF],
            )
            nc.scalar.dma_start(
                out=r_sb[:, 2 * TPB * CF :], in_=rr[:, 2 * TPB * CF :]
            )
            nc.scalar.dma_start(out=lp_sb[:, : 14 * P], in_=lp[:, : 14 * P])
            nc.scalar.dma_start(out=ln_sb[:, :], in_=ln[:, :])
            nc.scalar.dma_start(out=mk_sb[:, :], in_=mk[:, :])

            with (
                tc.tile_pool(name="psum", bufs=8, space="PSUM") as ppool,
                tc.tile_pool(name="out", bufs=16) as opool,
            ):
                # Greedy busy-ns balancing between the two drain engines
                # (Scalar also pays for the store dispatches on its ring).
                busy = {"v": 0.0, "s": 0.0}
                DRAIN_NS, DISPATCH_NS = 686.0, 610.0
                busy["s"] += 5 * DISPATCH_NS  # const loads above
                ti = 0   # store-ring toggle
                for t in reversed(range(NBLK)):
                    kc = t * P          # local row offset of this block
                    # Narrow tiles at the very start (stores begin sooner,
                    # rings ramp faster) and at the very end (shorter tail).
                    if t == NBLK - 1:
                        widths = (1, 1, 2, 4, 4, 4)
                    elif t == 0:
                        widths = (4, 4, 4, 2, 2)
                    else:
                        widths = (TPB,) * (NCHUNK // TPB)
                    cc0 = 0
                    for w in widths:
                        ot = opool.tile([P, w * CF], mybir.dt.float32)
                        for j in range(w):
                            cc = cc0 + j
                            pt = ppool.tile([P, CF], mybir.dt.float32)
                            lhs_sb = lp_sb if cc <= t else ln_sb
                            nc.tensor.matmul(
                                out=pt[:, :],
                                lhsT=lhs_sb[:, kc : kc + P],
                                rhs=r_sb[:, CF * cc : CF * (cc + 1)],
                                start=True,
                                stop=True,
                            )
                            dst = ot[:, CF * j : CF * (j + 1)]
                            if cc == t:
                                nc.vector.tensor_tensor(
                                    out=dst, in0=pt[:, :], in1=mk_sb[:, :], op=mult
                                )
                                busy["v"] += DRAIN_NS
                            elif busy["s"] <= busy["v"]:
                                nc.scalar.copy(out=dst, in_=pt[:, :])
                                busy["s"] += DRAIN_NS
                            else:
                                nc.vector.tensor_copy(out=dst, in_=pt[:, :])
                                busy["v"] += DRAIN_NS
                        if ti % 2 == 0:
                            ring = nc.sync
                        else:
                            ring = nc.scalar
                            busy["s"] += DISPATCH_NS
                        ring.dma_start(
                            out=out[kc : kc + P, CF * cc0 : CF * (cc0 + w)],
                            in_=ot[:, :],
                        )
                        ti += 1
                        cc0 += w
    nc.compile()
    return nc


def _get_program():
    if "nc" not in _PROGRAM_CACHE:
        _PROGRAM_CACHE["nc"] = _build_program()
    return _PROGRAM_CACHE["nc"]


def _host_tables(V):
    """Per-core input arrays."""
    import ml_dtypes

    bf16 = ml_dtypes.bfloat16

    pos = np.arange(N, dtype=np.float64)
    g = np.sqrt(np.abs(V).astype(np.float64) + EPS) / (1.0 + pos)  # (B, N)
    s2, c2 = np.sin(2.0 * pos), np.cos(2.0 * pos)

    def split(x):
        h = x.astype(bf16)
        l = (x - h.astype(np.float64)).astype(bf16)
        return h, l

    # Diagonal-band masks.  h=0 cores (even global blocks) have the band in
    # the first 128 complex cols of their diagonal chunk, h=1 in the second.
    p = np.arange(P)[:, None]
    jj = np.arange(P)[None, :]
    band = np.sign(p - jj).astype(np.float32)        # (128, 128)
    mask_e = np.empty((P, CF), dtype=np.float32)
    mask_o = np.empty((P, CF), dtype=np.float32)
    mask_e[:, 0:256:2] = band
    mask_e[:, 1:256:2] = band
    mask_e[:, 256:] = -1.0
    mask_o[:, 0:256] = 1.0
    mask_o[:, 256:512:2] = band
    mask_o[:, 257:512:2] = band

    in_maps = []
    for c in range(NCORES):
        b, h = divmod(c, 2)
        gb = g[b]
        # v-side basis, interleaved re/im, g_v folded in
        R0 = np.empty(2 * N); R1 = np.empty(2 * N)
        R0[0::2] = -gb * c2
        R0[1::2] = gb * s2
        R1[0::2] = gb * s2
        R1[1::2] = gb * c2
        R0h, R0l = split(R0)
        R1h, R1l = split(R1)
        rhs = np.ascontiguousarray(np.stack([R0h, R0l, R0h, R1h, R1l, R1h]))
        # u-side: this core's interleaved row blocks (global blocks 2t + h)
        ui = np.arange(N).reshape(2 * NBLK, P)[h::2].ravel()
        L0 = 0.5 * gb[ui] * s2[ui]
        L1 = 0.5 * gb[ui] * c2[ui]
        L0h, L0l = split(L0)
        L1h, L1l = split(L1)
        lpos = np.ascontiguousarray(np.stack([L0h, L0h, L0l, L1h, L1h, L1l]))
        lneg = np.ascontiguousarray(-lpos.astype(np.float32)).astype(bf16)
        in_maps.append(
            {
                "t_lp": lpos,
                "t_ln": lneg,
                "t_r": rhs,
                "t_mk": mask_e if h == 0 else mask_o,
            }
        )
    return in_maps


def _run(in_maps, trace=False, **kwargs):
    from concourse import bass_utils

    nc = _get_program()
    return bass_utils.run_bass_kernel_spmd(
        nc, in_maps, core_ids=list(range(NCORES)), trace=trace, **kwargs
    )


def kernel(V):
    V = np.asarray(V, dtype=np.float32)
    assert V.shape == (B, N), V.shape
    in_maps = _host_tables(V)
    res = _run(in_maps, trace=False)
    out = np.empty((B, N, N), dtype=np.complex64)
    for c in range(NCORES):
        b, h = divmod(c, 2)
        plane = np.ascontiguousarray(res.results[c]["t_out"])
        out[b].reshape(2 * NBLK, P, N)[h::2] = plane.view(np.complex64).reshape(
            NBLK, P, N
        )
    return out
